# revision 10
# baseline (speedup 1.0000x reference)
"""BitLinear (RMSNorm + int8-absmax activation quant + ternary weight quant
+ matmul) on 8 Trainium2 NeuronCores — v5.

Bit-faithful numerics (exact int8 activation values, exact global
a_scale/b_scale), restructured from v4 for steady-state overlap:

  - Activations are distributed as INT8 (not bf16): quantize -> bf16 z ->
    DMA-transpose -> int8 zt8 -> stage -> AllGather.  Halves both the
    collective payload (16MB -> 8MB) and the lhsT reload traffic.
  - AllGathers are split per (m-tile, k-half): 4 small AGs pipeline with
    the quantize/transpose stream and unblock the next rep's matmuls
    incrementally.
  - Software pipeline: the matmul phase of rep r-1 is emitted between the
    stats phase (S1) and quantize phase (S2) of rep r, so the ~105us PE
    stream overlaps the whole prologue.  DRAM staging buffers ping-pong by
    rep parity to kill cross-rep WAR serialization.
  - Ternary weights live in an 8-chunk bf16 tile ring: ternarize of rep r
    overlaps the matmul of rep r-1 chunk-by-chunk (GPSIMD), instead of
    serializing behind the whole phase (the main v4 stall).
  - W f32 quarters for rep r+1 prefetch-load at the end of iteration r.
  - int8 lhsT tiles convert to bf16 on ACT (h0,h1 g0-4) / GPSIMD (h1 g5-7);
    PSUM drain (x dequant scale) on ACT; DVE keeps stats + quantize.
  - rms_weight broadcast is loaded once, not per-rep.

Self-contained: only needs numpy + the platform's concourse/bass libraries.
"""

import os
import sys

import numpy as np

for _p in ("/opt/trn_rl_repo", "/root/.axon_site/_ro/trn_rl_repo"):
    if os.path.isdir(_p) and _p not in sys.path:
        sys.path.append(_p)

import concourse.bass as bass
import concourse.tile as tile
from concourse import mybir
from concourse.bass_utils import run_bass_kernel_spmd

R = 8  # cores
M, K, N = 2048, 4096, 4096
M_LOC = M // R  # 256 rows of x per core
N_LOC = N // R  # 512 weight columns per core
P = 128
KT = K // P  # 32 k-tiles
MT_LOC = M_LOC // P  # 2 m-tiles per core
CH = 4  # k-tiles per bt chunk / lhsT group
NCH = KT // CH  # 8 chunks
KH = K // 2  # half-k for transpose/stage/AG granularity
HT = KT // 2  # 16 k-tiles per half
WQ = KT // 4  # 8 k-tiles per W quarter
EPS_RMS = 1e-6
Q_CLIP = 1e-5
MAGIC = 12582912.0  # 1.5 * 2**23: (v + MAGIC) - MAGIC == round-to-nearest-even(v)
F32 = mybir.dt.float32
BF16 = mybir.dt.bfloat16
I8 = mybir.dt.int8
AX = mybir.AxisListType
ALU = mybir.AluOpType

ZT = KT * P * P  # 524288 elems: one m-tile's transposed activations
ZTH = ZT // 2  # one k-half of that


def _split_waits(nc, max_waits=1):
    """This toolchain rejects instructions with several semaphore waits
    ("Too many sync wait commands"). Hoist excess waits onto no-op
    instructions just before the offender on the same engine."""
    counter = 0
    for f in nc.m.functions:
        for blk in f.blocks:
            new_insts = []
            for inst in blk.instructions:
                si = getattr(inst, "sync_info", None)
                waits = list(si.on_wait) if si is not None and si.on_wait else []
                if len(waits) > max_waits:
                    excess = waits[: len(waits) - max_waits]
                    keep = waits[len(waits) - max_waits :]
                    for i in range(0, len(excess), max_waits):
                        counter += 1
                        nop = mybir.InstNoOp(
                            name=f"waitsplit_{counter}_{inst.name}", ins=[], outs=[]
                        )
                        nop.engine = inst.engine
                        nop.bass_nofuse = True
                        nop.sync_info = mybir.SyncInfo(
                            on_wait=list(excess[i : i + max_waits]), on_update=[]
                        )
                        new_insts.append(nop)
                    si.on_wait = keep
                    inst.sync_info = si
                new_insts.append(inst)
            blk.instructions[:] = new_insts


def _bcast_ap(ap, p):
    """Broadcast a 1-D DRAM AP across p partitions (step-0 partition axis)."""
    return bass.AP(tensor=ap.tensor, offset=ap.offset, ap=[[0, p]] + list(ap.ap))


class _Ctx:
    pass


def build_kernel(reps=1, mode=None):
    nc = bass.Bass(num_devices=R)
    c = _Ctx()
    c.nc = nc
    c.rg = [list(range(R))]

    c.x_in = nc.declare_dram_parameter("x_loc", [M_LOC, K], F32, isOutput=False)
    c.w_in = nc.declare_dram_parameter("w_loc", [K, N_LOC], F32, isOutput=False)
    c.rms_in = nc.declare_dram_parameter("rms_w", [K], F32, isOutput=False)
    c.out_ext = nc.declare_dram_parameter("out_loc", [M, N_LOC], F32, isOutput=True)

    # ping-pong DRAM staging by rep parity
    c.sb_loc = [nc.dram_tensor(f"sb_loc{i}", [P * 2], F32) for i in range(2)]
    c.sb_all = [
        nc.dram_tensor(f"sb_all{i}", [R * P * 2], F32, addr_space="Shared")
        for i in range(2)
    ]
    c.wsc_d = [nc.dram_tensor(f"wsc_d{i}", [P * 2], F32) for i in range(2)]
    c.z8_loc = [
        [
            [nc.dram_tensor(f"z8l_{i}{mt}{h}", [ZTH], I8) for h in range(2)]
            for mt in range(MT_LOC)
        ]
        for i in range(2)
    ]
    c.z8_all = [
        [
            [
                nc.dram_tensor(f"z8a_{i}{mt}{h}", [R * ZTH], I8, addr_space="Shared")
                for h in range(2)
            ]
            for mt in range(MT_LOC)
        ]
        for i in range(2)
    ]

    with tile.TileContext(nc) as tc:
        from contextlib import ExitStack

        ctxs = dict(
            wq_p=tc.tile_pool(name="wq", bufs=4),
            rms_p=tc.tile_pool(name="rmsp", bufs=1),
            bt_p=tc.tile_pool(name="btp", bufs=NCH),
            xz_p=tc.tile_pool(name="xz", bufs=2),
            zb_p=tc.tile_pool(name="zb", bufs=2),
            ztp_p=tc.tile_pool(name="ztp", bufs=2),
            zt8_p=tc.tile_pool(name="zt8", bufs=2),
            lhs8_p=tc.tile_pool(name="lhs8", bufs=2),
            lhsb_p=tc.tile_pool(name="lhsb", bufs=2),
            psum_p=tc.tile_pool(name="psum", bufs=8, space="PSUM"),
            out_p=tc.tile_pool(name="outp", bufs=4),
            st_p=tc.tile_pool(name="st", bufs=2),
            small_p=tc.tile_pool(name="small", bufs=1),
        )
        with ExitStack() as es:
            for k, v in ctxs.items():
                setattr(c, k, es.enter_context(v))

            c.eps_t = c.small_p.tile([P, 1], F32, tag="eps", name="eps")
            nc.vector.memset(c.eps_t, EPS_RMS)
            c.rms_b = c.rms_p.tile([P, K], F32, tag="rms", name="rms_b")
            nc.scalar.dma_start(c.rms_b[:], _bcast_ap(c.rms_in[:], P))

            state = None
            wq_next = emit_w_load(c, 0)  # W(0) quarters
            for rep in range(reps):
                pp = rep % 2
                if mode == "mm_loop" and state is not None:
                    emit_s3(c, state, rep)
                    continue
                wq_cur = wq_next
                # per-engine emission order matters (in-order queues):
                # sync: lhsT loads first, then S1 round-trip, then S2 stages
                # ACT:  x dma, conv-h0, sqrt, conv-h1, drains+out
                l8 = lb = None
                if state is not None and mode is None:
                    l8 = emit_s3_loads(c, state, rep)
                xf_tiles = emit_s1_x(c, rep)
                if l8 is not None:
                    lb = emit_s3_conv_h0(c, state, rep, l8)
                s1 = emit_s1_main(c, rep, pp, wq_cur, xf_tiles)
                if l8 is not None:
                    emit_s3_main(c, state, rep, l8, lb)
                state = emit_s2(c, rep, pp, s1, wq_cur,
                                skip_ag=(mode == "body_noag"))
                if rep + 1 < reps and mode != "mm_loop":
                    wq_next = emit_w_load(c, rep + 1)
            if mode is None and state is not None:
                emit_s3(c, state, reps)

    _split_waits(nc)
    return nc


def emit_w_load(c, rep):
    """Load W as 4 f32 quarter tiles (prefetched at the end of the previous
    iteration in steady state; pool WAR deps gate on ternarize progress)."""
    nc = c.nc
    wq = []
    for q in range(4):
        t = c.wq_p.tile([P, WQ, N_LOC], F32, tag="wq", name=f"wq_{rep}_{q}")
        eng = nc.sync if q % 2 == 0 else nc.scalar
        eng.dma_start(
            t[:],
            c.w_in[q * WQ * P : (q + 1) * WQ * P, :].rearrange(
                "(kt p) n -> p kt n", p=P
            ),
        )
        wq.append(t)
    return wq


def emit_s1_x(c, rep):
    """x m-tile loads (scalar queue; xf slots free after quantize p2 of r-1)."""
    nc = c.nc
    xf_tiles = []
    for mt in range(MT_LOC):
        xf = c.xz_p.tile([P, K], F32, tag="xf", name=f"xf_{rep}_{mt}")
        nc.scalar.dma_start(xf[:], c.x_in[mt * P : (mt + 1) * P, :])
        xf_tiles.append(xf)
    return xf_tiles


def emit_s1_main(c, rep, pp, wq, xf_tiles):
    """Stats + per-core scalar reduction + AllGather #1 + stp load."""
    nc = c.nc

    # |W| sums first on DVE (W was prefetched; ready at iter start)
    ws = c.st_p.tile([P, 4], F32, tag="ws", name=f"ws_{rep}")
    for q in range(4):
        nc.vector.tensor_reduce(
            out=ws[:, q : q + 1], in_=wq[q][:], axis=AX.XY, op=ALU.add,
            apply_absolute_value=True,
        )

    # per m-tile: moments -> r, x*rms, per-row absmax
    amax_mt = c.st_p.tile([P, MT_LOC], F32, tag="amx", name=f"amx_{rep}")
    r_tiles = []
    for mt in range(MT_LOC):
        xf = xf_tiles[mt]
        xg = xf[:].rearrange("p (g d) -> p g d", d=512)
        stats6 = c.st_p.tile([P, 8, 6], F32, tag="st6", name=f"st6_{rep}_{mt}")
        for g in range(8):
            nc.vector.bn_stats(out=stats6[:, g, :], in_=xg[:, g, :])
        mv = c.st_p.tile([P, 2], F32, tag="mv", name=f"mv_{rep}_{mt}")
        nc.vector.bn_aggr(out=mv, in_=stats6[:])
        msq = c.st_p.tile([P, 1], F32, tag=f"msq{mt}", name=f"msq_{rep}_{mt}")
        nc.vector.tensor_tensor(out=msq, in0=mv[:, 0:1], in1=mv[:, 0:1],
                                op=ALU.mult)
        nc.vector.tensor_tensor(out=msq, in0=msq, in1=mv[:, 1:2], op=ALU.add)
        r_t = c.st_p.tile([P, 1], F32, tag=f"rt{mt}", name=f"rt_{rep}_{mt}")
        nc.scalar.activation(
            out=r_t, in_=msq, func=mybir.ActivationFunctionType.Sqrt,
            bias=c.eps_t, scale=1.0,
        )
        nc.vector.reciprocal(out=r_t, in_=r_t)
        r_tiles.append(r_t)
        nc.vector.tensor_tensor(out=xf[:], in0=xf[:], in1=c.rms_b[:],
                                op=ALU.mult)
        amax_raw = c.st_p.tile([P, 1], F32, tag=f"amr{mt}",
                               name=f"amr_{rep}_{mt}")
        nc.vector.tensor_reduce(
            out=amax_raw, in_=xf[:], axis=AX.X, op=ALU.max,
            apply_absolute_value=True,
        )
        nc.vector.tensor_tensor(
            out=amax_mt[:, mt : mt + 1], in0=amax_raw, in1=r_t, op=ALU.mult
        )

    # per-core scalars -> partition reduce via DRAM round-trip -> AG1
    pr = c.st_p.tile([P, 2], F32, tag="pr", name=f"pr_{rep}")
    nc.vector.tensor_reduce(out=pr[:, 0:1], in_=amax_mt[:], axis=AX.X, op=ALU.max)
    nc.vector.tensor_reduce(out=pr[:, 1:2], in_=ws[:], axis=AX.X, op=ALU.add)
    nc.sync.dma_start(c.wsc_d[pp][:].rearrange("(p t) -> p t", p=P), pr[:])
    wscb = c.st_p.tile([P, P, 2], F32, tag="wscb", name=f"wscb_{rep}")
    nc.sync.dma_start(
        wscb[:],
        bass.AP(tensor=c.wsc_d[pp][:].tensor, offset=0, ap=[[0, P], [2, P], [1, 2]]),
    )
    pc = c.st_p.tile([P, 2], F32, tag="pc", name=f"pc_{rep}")
    nc.vector.tensor_reduce(
        out=pc[:, 0:1], in_=wscb[:, :, 0:1], axis=AX.XY, op=ALU.max
    )
    nc.vector.tensor_reduce(
        out=pc[:, 1:2], in_=wscb[:, :, 1:2], axis=AX.XY, op=ALU.add
    )
    nc.sync.dma_start(c.sb_loc[pp][:].rearrange("(p t) -> p t", p=P), pc[:])
    nc.gpsimd.collective_compute(
        "AllGather", ALU.bypass, replica_groups=c.rg,
        ins=[c.sb_loc[pp][:]], outs=[c.sb_all[pp][:]],
    )
    stp = c.st_p.tile([P, R, 2], F32, tag="stp", name=f"stp_{rep}")
    nc.sync.dma_start(
        stp[:],
        bass.AP(tensor=c.sb_all[pp][:].tensor, offset=0,
                ap=[[2, P], [P * 2, R], [1, 2]]),
    )

    s1 = _Ctx()
    s1.xf_tiles = xf_tiles
    s1.r_tiles = r_tiles
    s1.stp = stp
    return s1


def emit_s2(c, rep, pp, s1, wq, skip_ag=False):
    """Scales, quantize->transpose->int8 stage, AllGathers #2, ternarize."""
    nc = c.nc

    # global a_scale / b_scale (identical on every core)
    stp = s1.stp
    gmax = c.st_p.tile([P, 1], F32, tag="gmax", name=f"gmax_{rep}")
    nc.vector.tensor_reduce(out=gmax, in_=stp[:, :, 0:1], axis=AX.XY, op=ALU.max)
    nc.vector.tensor_scalar_max(out=gmax, in0=gmax, scalar1=Q_CLIP)
    a_s = c.st_p.tile([P, 1], F32, tag="as", name=f"as_{rep}")
    nc.vector.reciprocal(out=a_s, in_=gmax)
    nc.vector.tensor_scalar_mul(out=a_s, in0=a_s, scalar1=127.0)
    gsum = c.st_p.tile([P, 1], F32, tag="gsum", name=f"gsum_{rep}")
    nc.vector.tensor_reduce(out=gsum, in_=stp[:, :, 1:2], axis=AX.XY, op=ALU.add)
    nc.vector.tensor_scalar(
        out=gsum, in0=gsum, scalar1=1.0 / (K * N), scalar2=Q_CLIP,
        op0=ALU.mult, op1=ALU.max,
    )
    b_s = c.st_p.tile([P, 1], F32, tag="bs", name=f"bs_{rep}")
    nc.vector.reciprocal(out=b_s, in_=gsum)
    dq = c.st_p.tile([P, 1], F32, tag="dq", name=f"dq_{rep}")
    nc.vector.tensor_tensor(out=dq, in0=gmax, in1=gsum, op=ALU.mult)
    nc.vector.tensor_scalar_mul(out=dq, in0=dq, scalar1=1.0 / 127.0)

    # quantize + transpose + int8 stage, per (m-tile, k-half)
    staged = []  # (mt, hk) staging DMA emitted
    for mt in range(MT_LOC):
        xf = s1.xf_tiles[mt]
        rs = c.st_p.tile([P, 1], F32, tag=f"rs{mt}", name=f"rs_{rep}_{mt}")
        nc.vector.tensor_tensor(out=rs, in0=s1.r_tiles[mt], in1=a_s, op=ALU.mult)
        nc.vector.tensor_scalar(
            out=xf[:], in0=xf[:], scalar1=rs, scalar2=MAGIC,
            op0=ALU.mult, op1=ALU.add,
        )
        for hk in range(2):
            zbt = c.zb_p.tile([P, KH], BF16, tag="zb", name=f"zb_{rep}_{mt}{hk}")
            nc.vector.tensor_scalar(
                out=zbt[:], in0=xf[:, hk * KH : (hk + 1) * KH], scalar1=MAGIC,
                scalar2=None, op0=ALU.subtract,
            )
            ztp = c.ztp_p.tile([P, HT, P], BF16, tag="ztp",
                               name=f"ztp_{rep}_{mt}{hk}")
            nc.sync.dma_start_transpose(ztp[:], zbt[:])
            zt8 = c.zt8_p.tile([P, HT, P], I8, tag="zt8",
                               name=f"zt8_{rep}_{mt}{hk}")
            nc.vector.tensor_copy(zt8[:], ztp[:])
            nc.sync.dma_start(
                c.z8_loc[pp][mt][hk][:].rearrange("(p f) -> p f", p=P),
                zt8[:].rearrange("p a b -> p (a b)"),
            )
            staged.append((mt, hk))

    # gpsimd program: AGs interleaved with ternarize chunks so each AG
    # dispatches as soon as its staging lands while ternarize fills gaps.
    bt_chunks = [None] * NCH
    tern_order = [[0], [1, 2], [3, 4], [5, 6, 7]]  # between the 4 AGs

    def emit_tern(cc):
        q, o = cc // 2, cc % 2
        sl = wq[q][:, o * CH : (o + 1) * CH, :]
        nc.gpsimd.tensor_scalar(
            out=sl, in0=sl, scalar1=b_s[:, 0:1], scalar2=MAGIC,
            op0=ALU.mult, op1=ALU.add,
        )
        nc.gpsimd.tensor_scalar(
            out=sl, in0=sl, scalar1=MAGIC, scalar2=1.0,
            op0=ALU.subtract, op1=ALU.min,
        )
        btc = c.bt_p.tile([P, CH, N_LOC], BF16, tag="bt",
                          name=f"bt_{rep}_{cc}")
        nc.gpsimd.tensor_scalar(
            out=btc[:], in0=sl, scalar1=-1.0, scalar2=None, op0=ALU.max
        )
        bt_chunks[cc] = btc

    for i, (mt, hk) in enumerate(staged):
        for cc in tern_order[i]:
            emit_tern(cc)
        if not skip_ag:
            nc.gpsimd.collective_compute(
                "AllGather", ALU.bypass, replica_groups=c.rg,
                ins=[c.z8_loc[pp][mt][hk][:]], outs=[c.z8_all[pp][mt][hk][:]],
            )

    st = _Ctx()
    st.bt_chunks = bt_chunks
    st.dq = dq
    st.pp = pp
    return st


def emit_s3_loads(c, st, rep):
    """All lhsT int8 loads up front on the sync queue (data ready at iter
    start; must precede S1's round-trip DMAs which wait on stats)."""
    nc = c.nc
    pp = st.pp
    lhs8_tiles = {}
    for h in range(2):
        for g in range(NCH):
            hk, gg = g // (NCH // 2), g % (NCH // 2)
            t = c.lhs8_p.tile([P, R, CH * P], I8, tag="lhs8",
                              name=f"l8_{rep}_{h}{g}")
            nc.sync.dma_start(
                t[:],
                bass.AP(
                    tensor=c.z8_all[pp][h][hk][:].tensor,
                    offset=gg * CH * P,
                    ap=[[HT * P, P], [ZTH, R], [1, CH * P]],
                ),
            )
            lhs8_tiles[(h, g)] = t
    return lhs8_tiles


def _conv(c, st, rep, lhs8_tiles, h, g):
    t8 = lhs8_tiles[(h, g)]
    tb = c.lhsb_p.tile([P, R, CH * P], BF16, tag="lhsb",
                       name=f"lb_{rep}_{h}{g}")
    if h == 1 and g >= 5:
        nc = c.nc
        nc.gpsimd.tensor_copy(tb[:], t8[:])
    else:
        c.nc.scalar.copy(tb[:], t8[:])
    return tb


def emit_s3_conv_h0(c, st, rep, lhs8_tiles):
    """h0 bf16 converts on ACT, emitted right after the x-load dispatches so
    the PE can start immediately (before S1's sqrt ops on the ACT queue)."""
    lhsb = {}
    for g in range(NCH):
        lhsb[(0, g)] = _conv(c, st, rep, lhs8_tiles, 0, g)
    return lhsb


def emit_s3_main(c, st, rep, lhs8_tiles, lhsb):
    """Matmul phase consuming the previous rep's gathered int8 activations.

    half h = m-tile group (every rank's mt h); g = 4-kt chunk. bf16
    converts for h1 on ACT (g0-4) / GPSIMD (g5-7); drains+dequant on ACT;
    output stores on scalar queue."""
    nc = c.nc

    for g in range(5):
        lhsb[(1, g)] = _conv(c, st, rep, lhs8_tiles, 1, g)
    for g in range(5, NCH):
        lhsb[(1, g)] = _conv(c, st, rep, lhs8_tiles, 1, g)

    for h in range(2):
        psums = [
            c.psum_p.tile([P, N_LOC], F32, tag="ps", name=f"ps_{rep}_{h}_{i}")
            for i in range(R)
        ]
        for g in range(NCH):
            tb = lhsb[(h, g)]
            btc = st.bt_chunks[g]
            for kk in range(CH):
                for rr in range(R):
                    nc.tensor.matmul(
                        psums[rr][:],
                        tb[:, rr, kk * P : (kk + 1) * P],
                        btc[:, kk, :],
                        start=(g == 0 and kk == 0),
                        stop=(g == NCH - 1 and kk == CH - 1),
                    )
        for rr in range(R):
            o_t = c.out_p.tile([P, N_LOC], F32, tag="ot",
                               name=f"ot_{rep}_{h}_{rr}")
            nc.scalar.activation(
                out=o_t[:], in_=psums[rr][:],
                func=mybir.ActivationFunctionType.Copy,
                bias=0.0, scale=st.dq[:, 0:1],
            )
            gm = 2 * rr + h
            nc.scalar.dma_start(c.out_ext[gm * P : (gm + 1) * P, :], o_t[:])


def emit_s3(c, st, rep):
    """Non-pipelined full matmul phase (epilogue / mm_loop mode)."""
    l8 = emit_s3_loads(c, st, rep)
    lb = emit_s3_conv_h0(c, st, rep, l8)
    emit_s3_main(c, st, rep, l8, lb)


_CACHE = {}


def _get_nc():
    if "nc" not in _CACHE:
        _CACHE["nc"] = build_kernel()
    return _CACHE["nc"]


def make_in_maps(x, weight, rms_weight):
    x = np.ascontiguousarray(np.asarray(x, dtype=np.float32)).reshape(M, K)
    weight = np.asarray(weight, dtype=np.float32)
    rms_weight = np.ascontiguousarray(np.asarray(rms_weight, dtype=np.float32))
    return [
        {
            "x_loc": np.ascontiguousarray(x[c * M_LOC : (c + 1) * M_LOC]),
            "w_loc": np.ascontiguousarray(weight[:, c * N_LOC : (c + 1) * N_LOC]),
            "rms_w": rms_weight,
        }
        for c in range(R)
    ]


def assemble_out(results):
    out = np.concatenate([results[c]["out_loc"] for c in range(R)], axis=1)
    return out.reshape(1, M, N)


def kernel(x, weight, rms_weight):
    nc = _get_nc()
    in_maps = make_in_maps(x, weight, rms_weight)
    res = run_bass_kernel_spmd(nc, in_maps, core_ids=list(range(R)))
    return assemble_out(res.results)


# revision 18
# speedup vs baseline: 2.7333x; 2.7333x over previous
"""BitLinear (RMSNorm + int8-absmax activation quant + ternary weight quant
+ matmul) on 8 Trainium2 NeuronCores — v5.

Bit-faithful numerics (exact int8 activation values, exact global
a_scale/b_scale), restructured from v4 for steady-state overlap:

  - Activations are distributed as INT8: quantize -> bf16 z -> DMA-transpose
    -> SWDGE *casting* DMA (bf16->int8) stages to DRAM -> AllGather int8.
    Halves the collective payload and the lhsT reload traffic vs bf16.
  - lhsT tiles load via SWDGE casting DMAs (int8 DRAM -> bf16 SBUF), so the
    up-conversion costs no compute-engine time at all.
  - AllGathers split per (m-tile, k-half): 4 small AGs pipeline with the
    quantize/transpose stream and unblock the next rep's matmuls early.
  - Software pipeline: the matmul phase of rep r-1 overlaps the whole body
    of rep r; DRAM staging ping-pongs by rep parity to kill cross-rep WAR.
  - Ternarize: ACT computes t = bf16(w*b_s + 192) -- the bf16 RNE at the
    [128,256) binade rounds to integers exactly (and any |w*b_s|>2 stays
    beyond the clip), then DVE does (t-192) min 1 / max -1 into fp8 bt
    chunks (ternary is exact in fp8; mixed bf16 x fp8 matmul is exact).
    bt is an 8-chunk ring so ternarize(r) overlaps matmul(r-1) per-chunk.
  - |W| column sums on ACT via activation(Abs, accum_out).
  - W f32 quarters for rep r+1 prefetch-load at the end of iteration r;
    rms broadcast loads once.

Self-contained: only needs numpy + the platform's concourse/bass libraries.
"""

import os
import sys

import numpy as np

for _p in ("/opt/trn_rl_repo", "/root/.axon_site/_ro/trn_rl_repo"):
    if os.path.isdir(_p) and _p not in sys.path:
        sys.path.append(_p)

import concourse.bass as bass
import concourse.tile as tile
from concourse import mybir
from concourse.bass_utils import run_bass_kernel_spmd

R = 8  # cores
M, K, N = 2048, 4096, 4096
M_LOC = M // R  # 256 rows of x per core
N_LOC = N // R  # 512 weight columns per core
P = 128
KT = K // P  # 32 k-tiles
MT_LOC = M_LOC // P  # 2 m-tiles per core
CH = 4  # k-tiles per bt chunk / lhsT group
NCH = KT // CH  # 8 chunks
KH = K // 2  # half-k, transpose/stage/AG granularity
HT = KT // 2  # 16 k-tiles per half
WQN = KT // 4  # 8 k-tiles per W quarter
EPS_RMS = 1e-6
Q_CLIP = 1e-5
MAGIC = 12582912.0  # 1.5 * 2**23 (f32 round-to-nearest-even trick)
TMAGIC = 192.0  # 1.5 * 2**7: bf16 step-1 binade for the ternarize round
F32 = mybir.dt.float32
BF16 = mybir.dt.bfloat16
I8 = mybir.dt.int8
FP8 = mybir.dt.float8e4
AX = mybir.AxisListType
ALU = mybir.AluOpType
AF = mybir.ActivationFunctionType

ZT = KT * P * P  # 524288 elems: one m-tile's transposed activations
ZTH = ZT // 2  # one k-half of that


def _split_waits(nc, max_waits=1):
    """This toolchain rejects instructions with several semaphore waits
    ("Too many sync wait commands"). Hoist excess waits onto no-op
    instructions just before the offender on the same engine."""
    counter = 0
    for f in nc.m.functions:
        for blk in f.blocks:
            new_insts = []
            for inst in blk.instructions:
                si = getattr(inst, "sync_info", None)
                waits = list(si.on_wait) if si is not None and si.on_wait else []
                if len(waits) > max_waits:
                    excess = waits[: len(waits) - max_waits]
                    keep = waits[len(waits) - max_waits :]
                    for i in range(0, len(excess), max_waits):
                        counter += 1
                        nop = mybir.InstNoOp(
                            name=f"waitsplit_{counter}_{inst.name}", ins=[], outs=[]
                        )
                        nop.engine = inst.engine
                        nop.bass_nofuse = True
                        nop.sync_info = mybir.SyncInfo(
                            on_wait=list(excess[i : i + max_waits]), on_update=[]
                        )
                        new_insts.append(nop)
                    si.on_wait = keep
                    inst.sync_info = si
                new_insts.append(inst)
            blk.instructions[:] = new_insts


def _bcast_ap(ap, p):
    return bass.AP(tensor=ap.tensor, offset=ap.offset, ap=[[0, p]] + list(ap.ap))


class _Ctx:
    pass


def build_kernel(reps=1, mode=None):
    nc = bass.Bass(num_devices=R)
    c = _Ctx()
    c.nc = nc
    c.rg = [list(range(R))]

    c.x_in = nc.declare_dram_parameter("x_loc", [M_LOC, K], F32, isOutput=False)
    c.w_in = nc.declare_dram_parameter("w_loc", [K, N_LOC], F32, isOutput=False)
    c.rms_in = nc.declare_dram_parameter("rms_w", [K], F32, isOutput=False)
    c.out_ext = nc.declare_dram_parameter("out_loc", [M, N_LOC], F32, isOutput=True)

    c.sb_loc = [nc.dram_tensor(f"sb_loc{i}", [P * 2], F32) for i in range(2)]
    c.sb_all = [
        nc.dram_tensor(f"sb_all{i}", [R * P * 2], F32, addr_space="Shared")
        for i in range(2)
    ]
    c.wsc_d = [nc.dram_tensor(f"wsc_d{i}", [P * 2], F32) for i in range(2)]
    c.z8_loc = [
        [
            [nc.dram_tensor(f"z8l_{i}{mt}{h}", [ZTH], I8) for h in range(2)]
            for mt in range(MT_LOC)
        ]
        for i in range(2)
    ]
    c.z8_all = [
        [
            [
                nc.dram_tensor(f"z8a_{i}{mt}{h}", [R * ZTH], I8, addr_space="Shared")
                for h in range(2)
            ]
            for mt in range(MT_LOC)
        ]
        for i in range(2)
    ]

    with tile.TileContext(nc) as tc:
        from contextlib import ExitStack

        ctxs = dict(
            wq_p=tc.tile_pool(name="wq", bufs=4),
            rms_p=tc.tile_pool(name="rmsp", bufs=1),
            bt_p=tc.tile_pool(name="btp", bufs=NCH),
            tw_p=tc.tile_pool(name="twp", bufs=4),
            xz_p=tc.tile_pool(name="xz", bufs=2),
            zb_p=tc.tile_pool(name="zb", bufs=2),
            ztp_p=tc.tile_pool(name="ztp", bufs=2),
            lhsb_p=tc.tile_pool(name="lhsb", bufs=3),
            psum_p=tc.tile_pool(name="psum", bufs=8, space="PSUM"),
            out_p=tc.tile_pool(name="outp", bufs=2),
            st_p=tc.tile_pool(name="st", bufs=2),
            scr_p=tc.tile_pool(name="scr", bufs=1),
            small_p=tc.tile_pool(name="small", bufs=1),
        )
        with ExitStack() as es:
            for k, v in ctxs.items():
                setattr(c, k, es.enter_context(v))

            c.eps_t = c.small_p.tile([P, 1], F32, tag="eps", name="eps")
            nc.vector.memset(c.eps_t, EPS_RMS)
            c.rms_b = c.rms_p.tile([P, K], F32, tag="rms", name="rms_b")
            nc.scalar.dma_start(c.rms_b[:], _bcast_ap(c.rms_in[:], P))
            # wsum scratch (ACT Abs output target, per chunk-sized slab)
            c.wscr = c.scr_p.tile([P, CH, N_LOC], BF16, tag="wscr", name="wscr")

            state = None
            wq_next = emit_w_load(c, 0)
            rest = list(range(2, NCH)) + list(range(NCH, 2 * NCH))
            for rep in range(reps):
                pp = rep % 2
                if mode == "mm_loop" and state is not None:
                    emit_s3_loads(c, state, rep, range(2 * NCH))
                    emit_s3_half(c, state, rep, 0)
                    emit_s3_half(c, state, rep, 1)
                    continue
                wq_cur = wq_next
                prev = state
                if prev is not None and mode is None:
                    # lhsT casting loads: h0 g2..g7 + all h1 (h0 g0/g1 were
                    # prefetched at the end of the previous iteration)
                    emit_s3_loads(c, prev, rep, rest)
                xf_tiles = emit_s1_x(c, rep)
                s1 = emit_s1_main(c, rep, pp, wq_cur, xf_tiles)
                if prev is not None and mode is None:
                    emit_s3_half(c, prev, rep, 0)
                state = emit_s2(c, rep, pp, s1, wq_cur,
                                skip_ag=(mode == "body_noag"))
                if rep + 1 < reps and mode != "mm_loop":
                    wq_next = emit_w_load(c, rep + 1)
                if prev is not None and mode is None:
                    emit_s3_half(c, prev, rep, 1)
                # ternarize p2/p3 (and wave-B p1) AFTER all readers of the
                # previous rep's bt ring are emitted (WAR tracking)
                emit_tern23(c, state, rep)
                if mode is None:
                    emit_s3_loads(c, state, rep + 1, [0, 1])  # prefetch h0 g0/g1
            if mode is None and state is not None:
                emit_s3_loads(c, state, reps, rest)
                emit_s3_half(c, state, reps, 0)
                emit_s3_half(c, state, reps, 1)

    _split_waits(nc)
    return nc


def emit_w_load(c, rep):
    nc = c.nc
    wq = []
    for q in range(4):
        t = c.wq_p.tile([P, WQN, N_LOC], F32, tag="wq", name=f"wq_{rep}_{q}")
        eng = nc.sync if q % 2 == 0 else nc.scalar
        eng.dma_start(
            t[:],
            c.w_in[q * WQN * P : (q + 1) * WQN * P, :].rearrange(
                "(kt p) n -> p kt n", p=P
            ),
        )
        wq.append(t)
    return wq


def emit_s1_x(c, rep):
    nc = c.nc
    xf_tiles = []
    for mt in range(MT_LOC):
        xf = c.xz_p.tile([P, K], F32, tag="xf", name=f"xf_{rep}_{mt}")
        nc.scalar.dma_start(xf[:], c.x_in[mt * P : (mt + 1) * P, :])
        xf_tiles.append(xf)
    return xf_tiles


def emit_s1_main(c, rep, pp, wq, xf_tiles):
    """Stats (DVE) + wsum (ACT) + partition reduce + AllGather #1 + stp."""
    nc = c.nc

    # per m-tile: moments -> r, x*rms, per-row absmax (DVE; sqrt on ACT)
    amax_mt = c.st_p.tile([P, MT_LOC], F32, tag="amx", name=f"amx_{rep}")
    r_tiles = []
    for mt in range(MT_LOC):
        xf = xf_tiles[mt]
        xg = xf[:].rearrange("p (g d) -> p g d", d=512)
        stats6 = c.st_p.tile([P, 8, 6], F32, tag="st6", name=f"st6_{rep}_{mt}")
        for g in range(8):
            nc.vector.bn_stats(out=stats6[:, g, :], in_=xg[:, g, :])
        mv = c.st_p.tile([P, 2], F32, tag="mv", name=f"mv_{rep}_{mt}")
        nc.vector.bn_aggr(out=mv, in_=stats6[:])
        msq = c.st_p.tile([P, 1], F32, tag=f"msq{mt}", name=f"msq_{rep}_{mt}")
        nc.vector.tensor_tensor(out=msq, in0=mv[:, 0:1], in1=mv[:, 0:1],
                                op=ALU.mult)
        nc.vector.tensor_tensor(out=msq, in0=msq, in1=mv[:, 1:2], op=ALU.add)
        r_t = c.st_p.tile([P, 1], F32, tag=f"rt{mt}", name=f"rt_{rep}_{mt}")
        nc.scalar.activation(out=r_t, in_=msq, func=AF.Sqrt,
                             bias=c.eps_t, scale=1.0)
        nc.vector.reciprocal(out=r_t, in_=r_t)
        r_tiles.append(r_t)
        nc.vector.tensor_tensor(out=xf[:], in0=xf[:], in1=c.rms_b[:],
                                op=ALU.mult)
        amax_raw = c.st_p.tile([P, 1], F32, tag=f"amr{mt}",
                               name=f"amr_{rep}_{mt}")
        nc.vector.tensor_reduce(
            out=amax_raw, in_=xf[:], axis=AX.X, op=ALU.max,
            apply_absolute_value=True,
        )
        nc.vector.tensor_tensor(
            out=amax_mt[:, mt : mt + 1], in0=amax_raw, in1=r_t, op=ALU.mult
        )

    # |W| sums on ACT: Abs with accumulate, chunk-sized passes
    ws = c.st_p.tile([P, NCH], F32, tag="ws", name=f"ws_{rep}")
    for cc in range(NCH):
        q, o = cc // 2, cc % 2
        nc.scalar.activation(
            out=c.wscr[:], in_=wq[q][:, o * CH : (o + 1) * CH, :],
            func=AF.Abs, bias=0.0, scale=1.0,
            accum_out=ws[:, cc : cc + 1],
        )

    pr = c.st_p.tile([P, 2], F32, tag="pr", name=f"pr_{rep}")
    nc.vector.tensor_reduce(out=pr[:, 0:1], in_=amax_mt[:], axis=AX.X, op=ALU.max)
    nc.vector.tensor_reduce(out=pr[:, 1:2], in_=ws[:], axis=AX.X, op=ALU.add)
    nc.sync.dma_start(c.wsc_d[pp][:].rearrange("(p t) -> p t", p=P), pr[:])
    wscb = c.st_p.tile([P, P, 2], F32, tag="wscb", name=f"wscb_{rep}")
    nc.sync.dma_start(
        wscb[:],
        bass.AP(tensor=c.wsc_d[pp][:].tensor, offset=0, ap=[[0, P], [2, P], [1, 2]]),
    )
    pc = c.st_p.tile([P, 2], F32, tag="pc", name=f"pc_{rep}")
    nc.vector.tensor_reduce(
        out=pc[:, 0:1], in_=wscb[:, :, 0:1], axis=AX.XY, op=ALU.max
    )
    nc.vector.tensor_reduce(
        out=pc[:, 1:2], in_=wscb[:, :, 1:2], axis=AX.XY, op=ALU.add
    )
    nc.sync.dma_start(c.sb_loc[pp][:].rearrange("(p t) -> p t", p=P), pc[:])
    nc.gpsimd.collective_compute(
        "AllGather", ALU.bypass, replica_groups=c.rg,
        ins=[c.sb_loc[pp][:]], outs=[c.sb_all[pp][:]],
    )
    stp = c.st_p.tile([P, R, 2], F32, tag="stp", name=f"stp_{rep}")
    nc.sync.dma_start(
        stp[:],
        bass.AP(tensor=c.sb_all[pp][:].tensor, offset=0,
                ap=[[2, P], [P * 2, R], [1, 2]]),
    )

    s1 = _Ctx()
    s1.xf_tiles = xf_tiles
    s1.r_tiles = r_tiles
    s1.stp = stp
    return s1


def emit_s2(c, rep, pp, s1, wq, skip_ag=False):
    """Scales, quantize->transpose->casting int8 stage, AGs, ternarize."""
    nc = c.nc

    stp = s1.stp
    gmax = c.st_p.tile([P, 1], F32, tag="gmax", name=f"gmax_{rep}")
    nc.vector.tensor_reduce(out=gmax, in_=stp[:, :, 0:1], axis=AX.XY, op=ALU.max)
    nc.vector.tensor_scalar_max(out=gmax, in0=gmax, scalar1=Q_CLIP)
    a_s = c.st_p.tile([P, 1], F32, tag="as", name=f"as_{rep}")
    nc.vector.reciprocal(out=a_s, in_=gmax)
    nc.vector.tensor_scalar_mul(out=a_s, in0=a_s, scalar1=127.0)
    gsum = c.st_p.tile([P, 1], F32, tag="gsum", name=f"gsum_{rep}")
    nc.vector.tensor_reduce(out=gsum, in_=stp[:, :, 1:2], axis=AX.XY, op=ALU.add)
    nc.vector.tensor_scalar(
        out=gsum, in0=gsum, scalar1=1.0 / (K * N), scalar2=Q_CLIP,
        op0=ALU.mult, op1=ALU.max,
    )
    b_s = c.st_p.tile([P, 1], F32, tag="bs", name=f"bs_{rep}")
    nc.vector.reciprocal(out=b_s, in_=gsum)
    dq = c.st_p.tile([P, 1], F32, tag="dq", name=f"dq_{rep}")
    nc.vector.tensor_tensor(out=dq, in0=gmax, in1=gsum, op=ALU.mult)
    nc.vector.tensor_scalar_mul(out=dq, in0=dq, scalar1=1.0 / 127.0)

    # ternarize pass 1 (wave A: chunks 0-3) on ACT, bit-exact single-round:
    # wq <- w*b_s + MAGIC (f32, the add IS the RNE-to-integer), then
    # tw <- bf16(wq - MAGIC) (exact small integers)
    tw_tiles = {}

    def tern_p1(cc):
        q, o = cc // 2, cc % 2
        sl = wq[q][:, o * CH : (o + 1) * CH, :]
        nc.scalar.activation(
            out=sl, in_=sl, func=AF.Copy, bias=MAGIC, scale=b_s[:, 0:1],
        )
        tb = c.tw_p.tile([P, CH, N_LOC], BF16, tag="tw", name=f"tw_{rep}_{cc}")
        nc.scalar.activation(
            out=tb[:], in_=sl, func=AF.Copy, bias=-MAGIC, scale=1.0,
        )
        tw_tiles[cc] = tb

    for cc in range(NCH // 2):
        tern_p1(cc)

    # quantize + transpose + casting int8 stage, per (m-tile, k-half)
    for mt in range(MT_LOC):
        xf = s1.xf_tiles[mt]
        rs = c.st_p.tile([P, 1], F32, tag=f"rs{mt}", name=f"rs_{rep}_{mt}")
        nc.vector.tensor_tensor(out=rs, in0=s1.r_tiles[mt], in1=a_s, op=ALU.mult)
        nc.vector.tensor_scalar(
            out=xf[:], in0=xf[:], scalar1=rs, scalar2=MAGIC,
            op0=ALU.mult, op1=ALU.add,
        )
        for hk in range(2):
            zbt = c.zb_p.tile([P, KH], BF16, tag="zb", name=f"zb_{rep}_{mt}{hk}")
            nc.vector.tensor_scalar(
                out=zbt[:], in0=xf[:, hk * KH : (hk + 1) * KH], scalar1=MAGIC,
                scalar2=None, op0=ALU.subtract,
            )
            ztp = c.ztp_p.tile([P, HT, P], BF16, tag="ztp",
                               name=f"ztp_{rep}_{mt}{hk}")
            nc.sync.dma_start_transpose(ztp[:], zbt[:])
            # casting stage: bf16 SBUF -> int8 DRAM (SWDGE)
            nc.gpsimd.dma_start(
                c.z8_loc[pp][mt][hk][:].rearrange("(p f) -> p f", p=P),
                ztp[:].rearrange("p a b -> p (a b)"),
            )
            if not skip_ag:
                nc.gpsimd.collective_compute(
                    "AllGather", ALU.bypass, replica_groups=c.rg,
                    ins=[c.z8_loc[pp][mt][hk][:]],
                    outs=[c.z8_all[pp][mt][hk][:]],
                )

    st = _Ctx()
    st.bt_chunks = [None] * NCH
    st.tw_tiles = tw_tiles
    st.tern_p1 = tern_p1
    st.dq = dq
    st.pp = pp
    st.lhsb = {}
    return st


def emit_tern23(c, st, rep):
    """Ternarize passes 2+3 on DVE (and wave-B pass 1 on ACT) into the fp8
    bt chunk ring.  Emitted only after every reader of the previous rep's
    bt ring, so the pool's WAR deps are tracked in program order."""
    nc = c.nc

    def p23(cc):
        tb = st.tw_tiles[cc]
        btc = c.bt_p.tile([P, CH, N_LOC], FP8, tag="bt", name=f"bt_{rep}_{cc}")
        nc.vector.tensor_scalar(
            out=btc[:], in0=tb[:], scalar1=1.0, scalar2=-1.0,
            op0=ALU.min, op1=ALU.max,
        )
        st.bt_chunks[cc] = btc

    for cc in range(NCH // 2):
        p23(cc)
    for cc in range(NCH // 2, NCH):
        st.tern_p1(cc)
    for cc in range(NCH // 2, NCH):
        p23(cc)


def emit_s3_loads(c, st, rep, idxs):
    """lhsT casting loads (SWDGE: int8 DRAM -> bf16 SBUF), idx = h*NCH+g."""
    nc = c.nc
    pp = st.pp
    for idx in idxs:
        h, g = idx // NCH, idx % NCH
        hk, gg = g // (NCH // 2), g % (NCH // 2)
        t = c.lhsb_p.tile([P, R, CH * P], BF16, tag="lhsb",
                          name=f"lb_{rep}_{h}{g}")
        nc.gpsimd.dma_start(
            t[:],
            bass.AP(
                tensor=c.z8_all[pp][h][hk][:].tensor,
                offset=gg * CH * P,
                ap=[[HT * P, P], [ZTH, R], [1, CH * P]],
            ),
        )
        st.lhsb[(h, g)] = t


def emit_s3_half(c, st, rep, h):
    """One matmul half (m-tile group h): 8 groups x (4 kt x 8 ranks) into 8
    PSUM banks; ACT drains with dequant scale; scalar-queue out stores."""
    nc = c.nc
    lhsb = st.lhsb

    psums = [
        c.psum_p.tile([P, N_LOC], F32, tag="ps", name=f"ps_{rep}_{h}_{i}")
        for i in range(R)
    ]
    for g in range(NCH):
        tb = lhsb[(h, g)]
        btc = st.bt_chunks[g]
        for kk in range(CH):
            for rr in range(R):
                nc.tensor.matmul(
                    psums[rr][:],
                    tb[:, rr, kk * P : (kk + 1) * P],
                    btc[:, kk, :],
                    start=(g == 0 and kk == 0),
                    stop=(g == NCH - 1 and kk == CH - 1),
                )
    for rr in range(R):
        o_t = c.out_p.tile([P, N_LOC], F32, tag="ot",
                           name=f"ot_{rep}_{h}_{rr}")
        nc.scalar.activation(
            out=o_t[:], in_=psums[rr][:], func=AF.Copy,
            bias=0.0, scale=st.dq[:, 0:1],
        )
        gm = 2 * rr + h
        nc.scalar.dma_start(c.out_ext[gm * P : (gm + 1) * P, :], o_t[:])


_CACHE = {}


def _get_nc():
    if "nc" not in _CACHE:
        _CACHE["nc"] = build_kernel()
    return _CACHE["nc"]


def make_in_maps(x, weight, rms_weight):
    x = np.ascontiguousarray(np.asarray(x, dtype=np.float32)).reshape(M, K)
    weight = np.asarray(weight, dtype=np.float32)
    rms_weight = np.ascontiguousarray(np.asarray(rms_weight, dtype=np.float32))
    return [
        {
            "x_loc": np.ascontiguousarray(x[c * M_LOC : (c + 1) * M_LOC]),
            "w_loc": np.ascontiguousarray(weight[:, c * N_LOC : (c + 1) * N_LOC]),
            "rms_w": rms_weight,
        }
        for c in range(R)
    ]


def assemble_out(results):
    out = np.concatenate([results[c]["out_loc"] for c in range(R)], axis=1)
    return out.reshape(1, M, N)


def kernel(x, weight, rms_weight):
    nc = _get_nc()
    in_maps = make_in_maps(x, weight, rms_weight)
    res = run_bass_kernel_spmd(nc, in_maps, core_ids=list(range(R)))
    return assemble_out(res.results)


# revision 21
# speedup vs baseline: 2.7929x; 1.0218x over previous
"""BitLinear (RMSNorm + int8-absmax activation quant + ternary weight quant
+ matmul) on 8 Trainium2 NeuronCores — v5.

Bit-faithful numerics (exact int8 activation values, exact global
a_scale/b_scale), restructured from v4 for steady-state overlap:

  - Activations are distributed as INT8: quantize -> bf16 z -> DMA-transpose
    -> SWDGE *casting* DMA (bf16->int8) stages to DRAM -> AllGather int8.
    Halves the collective payload and the lhsT reload traffic vs bf16.
  - lhsT tiles load via SWDGE casting DMAs (int8 DRAM -> bf16 SBUF), so the
    up-conversion costs no compute-engine time at all.
  - AllGathers split per (m-tile, k-half): 4 small AGs pipeline with the
    quantize/transpose stream and unblock the next rep's matmuls early.
  - Software pipeline: the matmul phase of rep r-1 overlaps the whole body
    of rep r; DRAM staging ping-pongs by rep parity to kill cross-rep WAR.
  - Ternarize: ACT computes t = bf16(w*b_s + 192) -- the bf16 RNE at the
    [128,256) binade rounds to integers exactly (and any |w*b_s|>2 stays
    beyond the clip), then DVE does (t-192) min 1 / max -1 into fp8 bt
    chunks (ternary is exact in fp8; mixed bf16 x fp8 matmul is exact).
    bt is an 8-chunk ring so ternarize(r) overlaps matmul(r-1) per-chunk.
  - |W| column sums on ACT via activation(Abs, accum_out).
  - W f32 quarters for rep r+1 prefetch-load at the end of iteration r;
    rms broadcast loads once.

Self-contained: only needs numpy + the platform's concourse/bass libraries.
"""

import os
import sys

import numpy as np

for _p in ("/opt/trn_rl_repo", "/root/.axon_site/_ro/trn_rl_repo"):
    if os.path.isdir(_p) and _p not in sys.path:
        sys.path.append(_p)

import concourse.bass as bass
import concourse.tile as tile
from concourse import mybir
from concourse.bass_utils import run_bass_kernel_spmd

R = 8  # cores
M, K, N = 2048, 4096, 4096
M_LOC = M // R  # 256 rows of x per core
N_LOC = N // R  # 512 weight columns per core
P = 128
KT = K // P  # 32 k-tiles
MT_LOC = M_LOC // P  # 2 m-tiles per core
CH = 4  # k-tiles per bt chunk / lhsT group
NCH = KT // CH  # 8 chunks
KH = K // 2  # half-k, transpose/stage/AG granularity
HT = KT // 2  # 16 k-tiles per half
WQN = KT // 4  # 8 k-tiles per W quarter
EPS_RMS = 1e-6
Q_CLIP = 1e-5
MAGIC = 12582912.0  # 1.5 * 2**23 (f32 round-to-nearest-even trick)
TMAGIC = 192.0  # 1.5 * 2**7: bf16 step-1 binade for the ternarize round
F32 = mybir.dt.float32
BF16 = mybir.dt.bfloat16
I8 = mybir.dt.int8
FP8 = mybir.dt.float8e4
AX = mybir.AxisListType
ALU = mybir.AluOpType
AF = mybir.ActivationFunctionType

ZT = KT * P * P  # 524288 elems: one m-tile's transposed activations
ZTH = ZT // 2  # one k-half of that


def _split_waits(nc, max_waits=1):
    """This toolchain rejects instructions with several semaphore waits
    ("Too many sync wait commands"). Hoist excess waits onto no-op
    instructions just before the offender on the same engine."""
    counter = 0
    for f in nc.m.functions:
        for blk in f.blocks:
            new_insts = []
            for inst in blk.instructions:
                si = getattr(inst, "sync_info", None)
                waits = list(si.on_wait) if si is not None and si.on_wait else []
                if len(waits) > max_waits:
                    excess = waits[: len(waits) - max_waits]
                    keep = waits[len(waits) - max_waits :]
                    for i in range(0, len(excess), max_waits):
                        counter += 1
                        nop = mybir.InstNoOp(
                            name=f"waitsplit_{counter}_{inst.name}", ins=[], outs=[]
                        )
                        nop.engine = inst.engine
                        nop.bass_nofuse = True
                        nop.sync_info = mybir.SyncInfo(
                            on_wait=list(excess[i : i + max_waits]), on_update=[]
                        )
                        new_insts.append(nop)
                    si.on_wait = keep
                    inst.sync_info = si
                new_insts.append(inst)
            blk.instructions[:] = new_insts


def _bcast_ap(ap, p):
    return bass.AP(tensor=ap.tensor, offset=ap.offset, ap=[[0, p]] + list(ap.ap))


class _Ctx:
    pass


def build_kernel(reps=1, mode=None):
    nc = bass.Bass(num_devices=R)
    c = _Ctx()
    c.nc = nc
    c.rg = [list(range(R))]

    c.x_in = nc.declare_dram_parameter("x_loc", [M_LOC, K], F32, isOutput=False)
    c.w_in = nc.declare_dram_parameter("w_loc", [K, N_LOC], F32, isOutput=False)
    c.rms_in = nc.declare_dram_parameter("rms_w", [K], F32, isOutput=False)
    c.out_ext = nc.declare_dram_parameter("out_loc", [M, N_LOC], F32, isOutput=True)

    c.sb_loc = [nc.dram_tensor(f"sb_loc{i}", [P * 2], F32) for i in range(2)]
    c.sb_all = [
        nc.dram_tensor(f"sb_all{i}", [R * P * 2], F32, addr_space="Shared")
        for i in range(2)
    ]
    c.wsc_d = [nc.dram_tensor(f"wsc_d{i}", [P * 2], F32) for i in range(2)]
    c.z8_loc = [
        [nc.dram_tensor(f"z8l_{i}{mt}", [ZT], I8) for mt in range(MT_LOC)]
        for i in range(2)
    ]
    c.z8_all = [
        [
            nc.dram_tensor(f"z8a_{i}{mt}", [R * ZT], I8, addr_space="Shared")
            for mt in range(MT_LOC)
        ]
        for i in range(2)
    ]

    with tile.TileContext(nc) as tc:
        from contextlib import ExitStack

        ctxs = dict(
            wq_p=tc.tile_pool(name="wq", bufs=4),
            rms_p=tc.tile_pool(name="rmsp", bufs=1),
            bt_p=tc.tile_pool(name="btp", bufs=NCH),
            tw_p=tc.tile_pool(name="twp", bufs=4),
            xz_p=tc.tile_pool(name="xz", bufs=2),
            zb_p=tc.tile_pool(name="zb", bufs=2),
            ztp_p=tc.tile_pool(name="ztp", bufs=2),
            lhsb_p=tc.tile_pool(name="lhsb", bufs=3),
            psum_p=tc.tile_pool(name="psum", bufs=8, space="PSUM"),
            out_p=tc.tile_pool(name="outp", bufs=2),
            st_p=tc.tile_pool(name="st", bufs=2),
            scr_p=tc.tile_pool(name="scr", bufs=1),
            small_p=tc.tile_pool(name="small", bufs=1),
        )
        with ExitStack() as es:
            for k, v in ctxs.items():
                setattr(c, k, es.enter_context(v))

            c.eps_t = c.small_p.tile([P, 1], F32, tag="eps", name="eps")
            nc.vector.memset(c.eps_t, EPS_RMS)
            c.rms_b = c.rms_p.tile([P, K], F32, tag="rms", name="rms_b")
            nc.scalar.dma_start(c.rms_b[:], _bcast_ap(c.rms_in[:], P))
            # wsum scratch (ACT Abs output target, per chunk-sized slab)
            c.wscr = c.scr_p.tile([P, CH, N_LOC], BF16, tag="wscr", name="wscr")

            state = None
            wq_next = emit_w_load(c, 0)
            rest = list(range(2, NCH)) + list(range(NCH, 2 * NCH))
            for rep in range(reps):
                pp = rep % 2
                if mode == "mm_loop" and state is not None:
                    emit_s3_loads(c, state, rep, range(2 * NCH))
                    emit_s3_half(c, state, rep, 0)
                    emit_s3_half(c, state, rep, 1)
                    continue
                wq_cur = wq_next
                prev = state
                if prev is not None and mode is None:
                    # lhsT casting loads: h0 g2..g7 + all h1 (h0 g0/g1 were
                    # prefetched at the end of the previous iteration)
                    emit_s3_loads(c, prev, rep, rest)
                xf_tiles = emit_s1_x(c, rep)
                s1 = emit_s1_main(c, rep, pp, wq_cur, xf_tiles)
                if prev is not None and mode is None:
                    emit_s3_half(c, prev, rep, 0)
                state = emit_s2(c, rep, pp, s1, wq_cur,
                                skip_ag=(mode == "body_noag"))
                if rep + 1 < reps and mode != "mm_loop":
                    wq_next = emit_w_load(c, rep + 1)
                if prev is not None and mode is None:
                    emit_s3_half(c, prev, rep, 1)
                # ternarize p2/p3 (and wave-B p1) AFTER all readers of the
                # previous rep's bt ring are emitted (WAR tracking)
                emit_tern23(c, state, rep)
                if mode is None:
                    emit_s3_loads(c, state, rep + 1, [0, 1])  # prefetch h0 g0/g1
            if mode is None and state is not None:
                emit_s3_loads(c, state, reps, rest)
                emit_s3_half(c, state, reps, 0)
                emit_s3_half(c, state, reps, 1)

    _split_waits(nc)
    return nc


def emit_w_load(c, rep):
    nc = c.nc
    wq = []
    for q in range(4):
        t = c.wq_p.tile([P, WQN, N_LOC], F32, tag="wq", name=f"wq_{rep}_{q}")
        eng = nc.sync if q % 2 == 0 else nc.scalar
        eng.dma_start(
            t[:],
            c.w_in[q * WQN * P : (q + 1) * WQN * P, :].rearrange(
                "(kt p) n -> p kt n", p=P
            ),
        )
        wq.append(t)
    return wq


def emit_s1_x(c, rep):
    nc = c.nc
    xf_tiles = []
    for mt in range(MT_LOC):
        xf = c.xz_p.tile([P, K], F32, tag="xf", name=f"xf_{rep}_{mt}")
        nc.scalar.dma_start(xf[:], c.x_in[mt * P : (mt + 1) * P, :])
        xf_tiles.append(xf)
    return xf_tiles


def emit_s1_main(c, rep, pp, wq, xf_tiles):
    """Stats (DVE) + wsum (ACT) + partition reduce + AllGather #1 + stp."""
    nc = c.nc

    # per m-tile: moments -> r, x*rms, per-row absmax (DVE; sqrt on ACT)
    amax_mt = c.st_p.tile([P, MT_LOC], F32, tag="amx", name=f"amx_{rep}")
    r_tiles = []
    for mt in range(MT_LOC):
        xf = xf_tiles[mt]
        xg = xf[:].rearrange("p (g d) -> p g d", d=512)
        stats6 = c.st_p.tile([P, 8, 6], F32, tag="st6", name=f"st6_{rep}_{mt}")
        for g in range(8):
            nc.vector.bn_stats(out=stats6[:, g, :], in_=xg[:, g, :])
        mv = c.st_p.tile([P, 2], F32, tag="mv", name=f"mv_{rep}_{mt}")
        nc.vector.bn_aggr(out=mv, in_=stats6[:])
        msq = c.st_p.tile([P, 1], F32, tag=f"msq{mt}", name=f"msq_{rep}_{mt}")
        nc.vector.tensor_tensor(out=msq, in0=mv[:, 0:1], in1=mv[:, 0:1],
                                op=ALU.mult)
        nc.vector.tensor_tensor(out=msq, in0=msq, in1=mv[:, 1:2], op=ALU.add)
        r_t = c.st_p.tile([P, 1], F32, tag=f"rt{mt}", name=f"rt_{rep}_{mt}")
        nc.scalar.activation(out=r_t, in_=msq, func=AF.Sqrt,
                             bias=c.eps_t, scale=1.0)
        nc.vector.reciprocal(out=r_t, in_=r_t)
        r_tiles.append(r_t)
        nc.vector.tensor_tensor(out=xf[:], in0=xf[:], in1=c.rms_b[:],
                                op=ALU.mult)
        amax_raw = c.st_p.tile([P, 1], F32, tag=f"amr{mt}",
                               name=f"amr_{rep}_{mt}")
        nc.vector.tensor_reduce(
            out=amax_raw, in_=xf[:], axis=AX.X, op=ALU.max,
            apply_absolute_value=True,
        )
        nc.vector.tensor_tensor(
            out=amax_mt[:, mt : mt + 1], in0=amax_raw, in1=r_t, op=ALU.mult
        )

    # |W| sums on ACT: Abs with accumulate, chunk-sized passes
    ws = c.st_p.tile([P, NCH], F32, tag="ws", name=f"ws_{rep}")
    for cc in range(NCH):
        q, o = cc // 2, cc % 2
        nc.scalar.activation(
            out=c.wscr[:], in_=wq[q][:, o * CH : (o + 1) * CH, :],
            func=AF.Abs, bias=0.0, scale=1.0,
            accum_out=ws[:, cc : cc + 1],
        )

    pr = c.st_p.tile([P, 2], F32, tag="pr", name=f"pr_{rep}")
    nc.vector.tensor_reduce(out=pr[:, 0:1], in_=amax_mt[:], axis=AX.X, op=ALU.max)
    nc.vector.tensor_reduce(out=pr[:, 1:2], in_=ws[:], axis=AX.X, op=ALU.add)
    nc.sync.dma_start(c.wsc_d[pp][:].rearrange("(p t) -> p t", p=P), pr[:])
    wscb = c.st_p.tile([P, P, 2], F32, tag="wscb", name=f"wscb_{rep}")
    nc.sync.dma_start(
        wscb[:],
        bass.AP(tensor=c.wsc_d[pp][:].tensor, offset=0, ap=[[0, P], [2, P], [1, 2]]),
    )
    pc = c.st_p.tile([P, 2], F32, tag="pc", name=f"pc_{rep}")
    nc.vector.tensor_reduce(
        out=pc[:, 0:1], in_=wscb[:, :, 0:1], axis=AX.XY, op=ALU.max
    )
    nc.vector.tensor_reduce(
        out=pc[:, 1:2], in_=wscb[:, :, 1:2], axis=AX.XY, op=ALU.add
    )
    nc.sync.dma_start(c.sb_loc[pp][:].rearrange("(p t) -> p t", p=P), pc[:])
    nc.gpsimd.collective_compute(
        "AllGather", ALU.bypass, replica_groups=c.rg,
        ins=[c.sb_loc[pp][:]], outs=[c.sb_all[pp][:]],
    )
    stp = c.st_p.tile([P, R, 2], F32, tag="stp", name=f"stp_{rep}")
    nc.sync.dma_start(
        stp[:],
        bass.AP(tensor=c.sb_all[pp][:].tensor, offset=0,
                ap=[[2, P], [P * 2, R], [1, 2]]),
    )

    s1 = _Ctx()
    s1.xf_tiles = xf_tiles
    s1.r_tiles = r_tiles
    s1.stp = stp
    return s1


def emit_s2(c, rep, pp, s1, wq, skip_ag=False):
    """Scales, quantize->transpose->casting int8 stage, AGs, ternarize."""
    nc = c.nc

    stp = s1.stp
    gmax = c.st_p.tile([P, 1], F32, tag="gmax", name=f"gmax_{rep}")
    nc.vector.tensor_reduce(out=gmax, in_=stp[:, :, 0:1], axis=AX.XY, op=ALU.max)
    nc.vector.tensor_scalar_max(out=gmax, in0=gmax, scalar1=Q_CLIP)
    a_s = c.st_p.tile([P, 1], F32, tag="as", name=f"as_{rep}")
    nc.vector.reciprocal(out=a_s, in_=gmax)
    nc.vector.tensor_scalar_mul(out=a_s, in0=a_s, scalar1=127.0)
    gsum = c.st_p.tile([P, 1], F32, tag="gsum", name=f"gsum_{rep}")
    nc.vector.tensor_reduce(out=gsum, in_=stp[:, :, 1:2], axis=AX.XY, op=ALU.add)
    nc.vector.tensor_scalar(
        out=gsum, in0=gsum, scalar1=1.0 / (K * N), scalar2=Q_CLIP,
        op0=ALU.mult, op1=ALU.max,
    )
    b_s = c.st_p.tile([P, 1], F32, tag="bs", name=f"bs_{rep}")
    nc.vector.reciprocal(out=b_s, in_=gsum)
    dq = c.st_p.tile([P, 1], F32, tag="dq", name=f"dq_{rep}")
    nc.vector.tensor_tensor(out=dq, in0=gmax, in1=gsum, op=ALU.mult)
    nc.vector.tensor_scalar_mul(out=dq, in0=dq, scalar1=1.0 / 127.0)

    # ternarize pass 1 (wave A: chunks 0-3) on ACT, bit-exact single-round:
    # wq <- w*b_s + MAGIC (f32, the add IS the RNE-to-integer), then
    # tw <- bf16(wq - MAGIC) (exact small integers)
    tw_tiles = {}

    def tern_p1(cc):
        q, o = cc // 2, cc % 2
        sl = wq[q][:, o * CH : (o + 1) * CH, :]
        nc.scalar.activation(
            out=sl, in_=sl, func=AF.Copy, bias=MAGIC, scale=b_s[:, 0:1],
        )
        tb = c.tw_p.tile([P, CH, N_LOC], BF16, tag="tw", name=f"tw_{rep}_{cc}")
        nc.scalar.activation(
            out=tb[:], in_=sl, func=AF.Copy, bias=-MAGIC, scale=1.0,
        )
        tw_tiles[cc] = tb

    for cc in range(NCH // 2):
        tern_p1(cc)

    # quantize + transpose + casting int8 stage, per (m-tile, k-half)
    for mt in range(MT_LOC):
        xf = s1.xf_tiles[mt]
        rs = c.st_p.tile([P, 1], F32, tag=f"rs{mt}", name=f"rs_{rep}_{mt}")
        nc.vector.tensor_tensor(out=rs, in0=s1.r_tiles[mt], in1=a_s, op=ALU.mult)
        nc.vector.tensor_scalar(
            out=xf[:], in0=xf[:], scalar1=rs, scalar2=MAGIC,
            op0=ALU.mult, op1=ALU.add,
        )
        for hk in range(2):
            zbt = c.zb_p.tile([P, KH], BF16, tag="zb", name=f"zb_{rep}_{mt}{hk}")
            nc.vector.tensor_scalar(
                out=zbt[:], in0=xf[:, hk * KH : (hk + 1) * KH], scalar1=MAGIC,
                scalar2=None, op0=ALU.subtract,
            )
            ztp = c.ztp_p.tile([P, HT, P], BF16, tag="ztp",
                               name=f"ztp_{rep}_{mt}{hk}")
            nc.sync.dma_start_transpose(ztp[:], zbt[:])
            # casting stage: bf16 SBUF -> int8 DRAM (SWDGE), k-half slice
            nc.gpsimd.dma_start(
                c.z8_loc[pp][mt][:].rearrange("(p f) -> p f", p=P)[
                    :, hk * (HT * P) : (hk + 1) * (HT * P)
                ],
                ztp[:].rearrange("p a b -> p (a b)"),
            )
        if not skip_ag:
            nc.gpsimd.collective_compute(
                "AllGather", ALU.bypass, replica_groups=c.rg,
                ins=[c.z8_loc[pp][mt][:]],
                outs=[c.z8_all[pp][mt][:]],
            )

    st = _Ctx()
    st.bt_chunks = [None] * NCH
    st.tw_tiles = tw_tiles
    st.tern_p1 = tern_p1
    st.dq = dq
    st.pp = pp
    st.lhsb = {}
    return st


def emit_tern23(c, st, rep):
    """Ternarize passes 2+3 on DVE (and wave-B pass 1 on ACT) into the fp8
    bt chunk ring.  Emitted only after every reader of the previous rep's
    bt ring, so the pool's WAR deps are tracked in program order."""
    nc = c.nc

    def p23(cc):
        tb = st.tw_tiles[cc]
        btc = c.bt_p.tile([P, CH, N_LOC], FP8, tag="bt", name=f"bt_{rep}_{cc}")
        nc.vector.tensor_scalar(
            out=btc[:], in0=tb[:], scalar1=1.0, scalar2=-1.0,
            op0=ALU.min, op1=ALU.max,
        )
        st.bt_chunks[cc] = btc

    for cc in range(NCH // 2):
        p23(cc)
    for cc in range(NCH // 2, NCH):
        st.tern_p1(cc)
    for cc in range(NCH // 2, NCH):
        p23(cc)


def emit_s3_loads(c, st, rep, idxs):
    """lhsT casting loads (SWDGE: int8 DRAM -> bf16 SBUF), idx = h*NCH+g."""
    nc = c.nc
    pp = st.pp
    for idx in idxs:
        h, g = idx // NCH, idx % NCH
        t = c.lhsb_p.tile([P, R, CH * P], BF16, tag="lhsb",
                          name=f"lb_{rep}_{h}{g}")
        nc.gpsimd.dma_start(
            t[:],
            bass.AP(
                tensor=c.z8_all[pp][h][:].tensor,
                offset=g * CH * P,
                ap=[[KT * P, P], [ZT, R], [1, CH * P]],
            ),
        )
        st.lhsb[(h, g)] = t


def emit_s3_half(c, st, rep, h):
    """One matmul half (m-tile group h): 8 groups x (4 kt x 8 ranks) into 8
    PSUM banks; ACT drains with dequant scale; scalar-queue out stores."""
    nc = c.nc
    lhsb = st.lhsb

    psums = [
        c.psum_p.tile([P, N_LOC], F32, tag="ps", name=f"ps_{rep}_{h}_{i}")
        for i in range(R)
    ]
    for g in range(NCH):
        tb = lhsb[(h, g)]
        btc = st.bt_chunks[g]
        for kk in range(CH):
            for rr in range(R):
                nc.tensor.matmul(
                    psums[rr][:],
                    tb[:, rr, kk * P : (kk + 1) * P],
                    btc[:, kk, :],
                    start=(g == 0 and kk == 0),
                    stop=(g == NCH - 1 and kk == CH - 1),
                )
    for rr in range(R):
        o_t = c.out_p.tile([P, N_LOC], F32, tag="ot",
                           name=f"ot_{rep}_{h}_{rr}")
        nc.scalar.activation(
            out=o_t[:], in_=psums[rr][:], func=AF.Copy,
            bias=0.0, scale=st.dq[:, 0:1],
        )
        gm = 2 * rr + h
        nc.scalar.dma_start(c.out_ext[gm * P : (gm + 1) * P, :], o_t[:])


_CACHE = {}


def _get_nc():
    if "nc" not in _CACHE:
        _CACHE["nc"] = build_kernel()
    return _CACHE["nc"]


def make_in_maps(x, weight, rms_weight):
    x = np.ascontiguousarray(np.asarray(x, dtype=np.float32)).reshape(M, K)
    weight = np.asarray(weight, dtype=np.float32)
    rms_weight = np.ascontiguousarray(np.asarray(rms_weight, dtype=np.float32))
    return [
        {
            "x_loc": np.ascontiguousarray(x[c * M_LOC : (c + 1) * M_LOC]),
            "w_loc": np.ascontiguousarray(weight[:, c * N_LOC : (c + 1) * N_LOC]),
            "rms_w": rms_weight,
        }
        for c in range(R)
    ]


def assemble_out(results):
    out = np.concatenate([results[c]["out_loc"] for c in range(R)], axis=1)
    return out.reshape(1, M, N)


def kernel(x, weight, rms_weight):
    nc = _get_nc()
    in_maps = make_in_maps(x, weight, rms_weight)
    res = run_bass_kernel_spmd(nc, in_maps, core_ids=list(range(R)))
    return assemble_out(res.results)
